# revision 14
# baseline (speedup 1.0000x reference)
"""Trainium2 Bass kernel for nn_DeltaNetLayer (B=4, L=1024, D=256).

Sharding: 8 cores = batch(4) x output-dim-half(2); the delta-rule state W
splits cleanly by output rows (v_bar = W @ phi_k needs only W's own rows).
Each core:
  - projects q,k (full D), its v half, and beta from x_b (fed transposed)
  - phi = LN(elu(.)+1) token-major, PE-transposed per chunk to feature-major
  - chunked recurrence (C=128, 8 chunks): per chunk the unit-lower-triangular
    system (I + diag(b) Gamma.G) Dm = b.(V - df^i K W0^T) is solved exactly
    with the nilpotent product-form inverse J = (I-N)(I+N^2)(I+N^4)(I+N^8)(I+N^16)
    (J is built off the sequential critical path; the carried chain per chunk
    is only KW0 -> rhs -> J@rhs -> W update)
  - final LN over full D needs both halves' stats: pairwise AllReduce of
    per-token (sum, sumsq), then each core emits a partial output projection
    y_norm[:, half] @ Wo[:, half].T
Host sums the two partials per batch element and adds bo.
"""

import numpy as np

import concourse.bass as bass
import concourse.bacc as bacc
import concourse.mybir as mybir
import concourse.tile as tile
from concourse.bass_utils import run_bass_kernel_spmd

B, L, D = 4, 1024, 256
R = 128           # output-dim rows per core
C = 128           # chunk length (tokens)
NCH = L // C      # 8 chunks
KT = D // 128     # 2 contraction tiles over D
LN_EPS = 1e-5
FP = mybir.dt.float32
ALU = mybir.AluOpType
AF = mybir.ActivationFunctionType
AX = mybir.AxisListType.X

REPLICA_GROUPS = [[0, 1], [2, 3], [4, 5], [6, 7]]

# extra kwargs for run_bass_kernel_spmd (test harness sets trace=True here)
_RUN_KWARGS = {}
_last_results = None


def _host_consts(df, Wq, Wk, beta_w, lnp_g, lnp_b):
    i = np.arange(C)
    pw = i[:, None] - 1 - i[None, :]
    gam = np.where(pw >= 0, df ** np.maximum(pw, 0), 0.0).astype(np.float32)
    consts = {
        "gam": gam,
        "gamT": np.ascontiguousarray(gam.T),
        "ident": np.eye(C, dtype=np.float32),
        # df^i broadcast along partitions (for free-dim scaling of phi_qT)
        "dfB": np.broadcast_to((df ** i).astype(np.float32), (128, C)).copy(),
        # per-partition column vectors: df^i, df^(C-1-i), -df^i
        "dfvec": np.stack(
            [df ** i, df ** (C - 1 - i), -(df ** i)], axis=1
        ).astype(np.float32),
        "wqT": np.ascontiguousarray(Wq.T.reshape(KT, 128, D).transpose(1, 0, 2)).astype(np.float32),
        "wkT": np.ascontiguousarray(Wk.T.reshape(KT, 128, D).transpose(1, 0, 2)).astype(np.float32),
        "bwT": np.ascontiguousarray(beta_w.T.reshape(KT, 128, 1).transpose(1, 0, 2)).astype(np.float32),
        "lnp": np.stack([lnp_g, lnp_b]).astype(np.float32),  # [2, D]
    }
    return consts


def _bcast_ap(src_ap, parts=128):
    """Broadcast a [1, N]-ish AP along the partition dim with stride 0."""
    return bass.AP(
        tensor=src_ap.tensor,
        offset=src_ap.offset,
        ap=[[0, parts], list(src_ap.ap[-1])],
    )


def _build(df, dfC, beta_b, consts):
    nc = bacc.Bacc(
        "TRN2",
        target_bir_lowering=False,
        debug=False,
        num_devices=2 * B,
    )

    # per-core I/O
    xT_d = nc.dram_tensor("xT", [128, KT, L], FP, kind="ExternalInput")
    wvT_d = nc.dram_tensor("wvT", [128, KT, R], FP, kind="ExternalInput")
    woT_d = nc.dram_tensor("woT", [R, D], FP, kind="ExternalInput")
    lngb_d = nc.dram_tensor("lngb", [2, R], FP, kind="ExternalInput")
    out_d = nc.dram_tensor("out_part", [L, D], FP, kind="ExternalOutput")

    # shared constants, baked into the NEFF
    gam_d = nc.inline_tensor(consts["gam"], "c_gam")
    gamT_d = nc.inline_tensor(consts["gamT"], "c_gamT")
    ident_d = nc.inline_tensor(consts["ident"], "c_ident")
    dfB_d = nc.inline_tensor(consts["dfB"], "c_dfB")
    dfvec_d = nc.inline_tensor(consts["dfvec"], "c_dfvec")
    wqT_d = nc.inline_tensor(consts["wqT"], "c_wqT")
    wkT_d = nc.inline_tensor(consts["wkT"], "c_wkT")
    bwT_d = nc.inline_tensor(consts["bwT"], "c_bwT")
    lnp_d = nc.inline_tensor(consts["lnp"], "c_lnp")

    with tile.TileContext(nc) as tc:
        with (
            tc.tile_pool(name="const", bufs=1) as pc,
            tc.tile_pool(name="pers", bufs=1) as pp,
            tc.tile_pool(name="scr", bufs=3) as ps,
            tc.tile_pool(name="scr2", bufs=2) as ps2,
            tc.tile_pool(name="psproj", bufs=2, space="PSUM") as ppj,
            tc.tile_pool(name="psprep", bufs=3, space="PSUM") as ppr,
            tc.tile_pool(name="pschain", bufs=3, space="PSUM") as pch,
            tc.tile_pool(name="dram", bufs=1, space="DRAM") as pd,
        ):
            # ---------------- constant / weight loads ----------------
            def ctile(nm, shape, src):
                t = pc.tile(shape, FP, name=nm)
                nc.gpsimd.dma_start(out=t[:], in_=src)
                return t

            gam = ctile("gam", [C, C], gam_d[:, :])
            gamT = ctile("gamT", [C, C], gamT_d[:, :])
            ident = ctile("ident", [C, C], ident_d[:, :])
            dfB = ctile("dfB", [128, C], dfB_d[:, :])
            dfvec = ctile("dfvec", [128, 3], dfvec_d[:, :])
            wq = ctile("wq", [128, KT, D], wqT_d[:, :, :])
            wk = ctile("wk", [128, KT, D], wkT_d[:, :, :])
            bw = ctile("bw", [128, KT, 1], bwT_d[:, :, :])
            wv = ctile("wv", [128, KT, R], wvT_d[:, :, :])
            wo = ctile("wo", [R, D], woT_d[:, :])
            lnpg = ctile("lnpg", [128, D], _bcast_ap(lnp_d[0, :]))
            lnpb = ctile("lnpb", [128, D], _bcast_ap(lnp_d[1, :]))
            lngB = ctile("lngB", [128, R], _bcast_ap(lngb_d[0, :]))
            lnbB = ctile("lnbB", [128, R], _bcast_ap(lngb_d[1, :]))
            eps_t = pc.tile([128, 1], FP)
            nc.vector.memset(eps_t[:], LN_EPS)
            bb_t = pc.tile([128, 1], FP)
            nc.vector.memset(bb_t[:], float(beta_b))
            xt = pc.tile([128, KT, L], FP)
            for c in range(NCH):
                nc.gpsimd.dma_start(
                    out=xt[:, :, c * C:(c + 1) * C],
                    in_=xT_d[:, :, c * C:(c + 1) * C],
                )

            # ---------------- persistent per-chunk storage ----------------
            phiq = pp.tile([128, NCH, D], FP)     # token-major phi_q
            phik = pp.tile([128, NCH, D], FP)
            phiqT = pp.tile([128, KT, L], FP)     # feature-major
            phikT = pp.tile([128, KT, L], FP)
            phiqTs = pp.tile([128, KT, L], FP)    # df^i-scaled feature-major q
            kps = pp.tile([128, NCH, D], FP)      # df^(C-1-i)-scaled k, token-major
            vv = pp.tile([128, NCH, R], FP)
            bV = pp.tile([128, NCH, R], FP)
            bcol = pp.tile([128, NCH], FP)        # beta
            nbdf = pp.tile([128, NCH], FP)        # -beta*df^i
            JTs = pp.tile([128, NCH, C], FP)
            ATs = pp.tile([128, NCH, C], FP)
            ys = pp.tile([128, NCH, R], FP)
            s1 = pp.tile([128, NCH], FP)
            s2 = pp.tile([128, NCH], FP)
            w_state = [pp.tile([128, KT, R], FP, name=f"w{i}")
                       for i in range(2)]

            def csl(c):
                return slice(c * C, (c + 1) * C)

            # ---------------- phase emitters ----------------
            def proj_chunk(c):
                sl = csl(c)
                # q and k projections + phi pipeline
                for w_sb, dst in ((wq, phiq), (wk, phik)):
                    pj = ppj.tile([128, D], FP, tag="proj")
                    nc.tensor.matmul(pj[:], lhsT=xt[:, 0, sl], rhs=w_sb[:, 0, :],
                                     start=True, stop=False)
                    nc.tensor.matmul(pj[:], lhsT=xt[:, 1, sl], rhs=w_sb[:, 1, :],
                                     start=False, stop=True)
                    # phi = LN(relu(x) + exp(min(x,0)))
                    e_t = ps.tile([128, D], FP, tag="elu_e")
                    r_t = ps.tile([128, D], FP, tag="elu_r")
                    nc.vector.tensor_scalar_min(e_t[:], pj[:], 0.0)
                    nc.scalar.activation(e_t[:], e_t[:], AF.Exp)
                    nc.vector.tensor_scalar_max(r_t[:], pj[:], 0.0)
                    nc.vector.tensor_add(r_t[:], r_t[:], e_t[:])
                    st6 = ps.tile([128, 6], FP, tag="st6")
                    mv = ps.tile([128, 2], FP, tag="mv")
                    sd = ps.tile([128, 1], FP, tag="sd")
                    rsd = ps.tile([128, 1], FP, tag="rsd")
                    nc.vector.bn_stats(out=st6[:], in_=r_t[:])
                    nc.vector.bn_aggr(out=mv[:], in_=st6[:])
                    nc.scalar.activation(sd[:], mv[:, 1:2], AF.Sqrt, bias=eps_t[:])
                    nc.vector.reciprocal(rsd[:], sd[:])
                    nc.vector.tensor_scalar(
                        out=r_t[:], in0=r_t[:], scalar1=mv[:, 0:1], scalar2=rsd[:],
                        op0=ALU.subtract, op1=ALU.mult)
                    nc.vector.tensor_mul(r_t[:], r_t[:], lnpg[:])
                    nc.vector.tensor_add(dst[:, c, :], r_t[:], lnpb[:])
                # v half
                pv = ppj.tile([128, R], FP, tag="proj")
                nc.tensor.matmul(pv[:], lhsT=xt[:, 0, sl], rhs=wv[:, 0, :],
                                 start=True, stop=False)
                nc.tensor.matmul(pv[:], lhsT=xt[:, 1, sl], rhs=wv[:, 1, :],
                                 start=False, stop=True)
                nc.vector.tensor_copy(vv[:, c, :], pv[:])
                # beta
                pb = ppj.tile([128, 1], FP, tag="proj")
                nc.tensor.matmul(pb[:], lhsT=xt[:, 0, sl], rhs=bw[:, 0, :],
                                 start=True, stop=False)
                nc.tensor.matmul(pb[:], lhsT=xt[:, 1, sl], rhs=bw[:, 1, :],
                                 start=False, stop=True)
                nc.scalar.activation(bcol[:, c:c + 1], pb[:], AF.Sigmoid,
                                     bias=bb_t[:])
                nc.vector.tensor_mul(nbdf[:, c:c + 1], bcol[:, c:c + 1],
                                     dfvec[:, 2:3])
                nc.vector.tensor_scalar_mul(bV[:, c, :], vv[:, c, :],
                                            bcol[:, c:c + 1])
                # df^(C-1-i)-scaled k (token-major, lhsT for the W update)
                nc.vector.tensor_scalar_mul(kps[:, c, :], phik[:, c, :],
                                            dfvec[:, 1:2])

            def prep_chunk(c):
                sl = csl(c)
                # transposes: phi_q, phi_k -> feature-major
                for src, dstT in ((phiq, phiqT), (phik, phikT)):
                    for kt in range(KT):
                        pt = ppr.tile([128, 128], FP, tag="prep")
                        nc.tensor.transpose(pt[:], src[:, c, kt * 128:(kt + 1) * 128],
                                            ident[:])
                        nc.vector.tensor_copy(dstT[:, kt, sl], pt[:])
                # scaled q^T (free-dim df^i scaling)
                for kt in range(KT):
                    nc.vector.tensor_mul(phiqTs[:, kt, sl], phiqT[:, kt, sl], dfB[:])
                # G = K K^T
                pg = ppr.tile([128, C], FP, tag="prep")
                nc.tensor.matmul(pg[:], lhsT=phikT[:, 0, sl], rhs=phikT[:, 0, sl],
                                 start=True, stop=False)
                nc.tensor.matmul(pg[:], lhsT=phikT[:, 1, sl], rhs=phikT[:, 1, sl],
                                 start=False, stop=True)
                # N = b_i * Gamma_ij * G_ij   (strictly lower)
                n_t = ps2.tile([128, C], FP, tag="n")
                nt_t = ps2.tile([128, C], FP, tag="nt")
                nc.vector.tensor_mul(n_t[:], pg[:], gam[:])
                nc.vector.tensor_scalar_mul(n_t[:], n_t[:], bcol[:, c:c + 1])
                ptr = ppr.tile([128, C], FP, tag="prep")
                nc.tensor.transpose(ptr[:], n_t[:], ident[:])
                nc.vector.tensor_copy(nt_t[:], ptr[:])
                # squarings: S2,S4,S8,S16 (+ transposed mates), A = -N
                # J = (I+A)(I+A^2)(I+A^4)(I+A^8)(I+A^16), A = -N (nilpotent)
                # JT built left-multiplying: JT += (A^2k)^T @ JT via lhsT=A^2k
                jt_cur = ps2.tile([128, C], FP, tag="jt")
                nc.vector.tensor_sub(jt_cur[:], ident[:], nt_t[:])
                s_cur, st_cur = n_t, nt_t  # holds N^(2^k); sign handled below
                for lvl in range(4):
                    # square: S2 = S@S (lhsT=ST), ST2 = ST@ST... = (S@S)^T (lhsT=S)
                    ps_a = ppr.tile([128, C], FP, tag="prep")
                    nc.tensor.matmul(ps_a[:], lhsT=st_cur[:], rhs=s_cur[:],
                                     start=True, stop=True)
                    s_new = ps2.tile([128, C], FP, tag=f"s{lvl}")
                    nc.vector.tensor_copy(s_new[:], ps_a[:])
                    if lvl < 3:
                        ps_b = ppr.tile([128, C], FP, tag="prep")
                        nc.tensor.matmul(ps_b[:], lhsT=s_cur[:], rhs=st_cur[:],
                                         start=True, stop=True)
                        st_new = ps2.tile([128, C], FP, tag=f"st{lvl}")
                        nc.vector.tensor_copy(st_new[:], ps_b[:])
                    else:
                        st_new = None
                    # note: powers of (-N): even powers = N^2k, so factors
                    # (I + A^2k) with A=-N equal (I + N^2k) for k>=1. The only
                    # sign flip is the first factor (I - N), already applied.
                    pj_f = ppr.tile([128, C], FP, tag="prep")
                    nc.tensor.matmul(pj_f[:], lhsT=s_new[:], rhs=jt_cur[:],
                                     start=True, stop=True)
                    if lvl < 3:
                        jt_new = ps2.tile([128, C], FP, tag=f"jt{lvl}")
                        nc.vector.tensor_add(jt_new[:], jt_cur[:], pj_f[:])
                        jt_cur = jt_new
                        s_cur, st_cur = s_new, st_new
                    else:
                        nc.vector.tensor_add(JTs[:, c, :], jt_cur[:], pj_f[:])
                # AT = (K Q^T) o Gamma^T
                pa = ppr.tile([128, C], FP, tag="prep")
                nc.tensor.matmul(pa[:], lhsT=phikT[:, 0, sl], rhs=phiqT[:, 0, sl],
                                 start=True, stop=False)
                nc.tensor.matmul(pa[:], lhsT=phikT[:, 1, sl], rhs=phiqT[:, 1, sl],
                                 start=False, stop=True)
                nc.vector.tensor_mul(ATs[:, c, :], pa[:], gamT[:])

            def chain_chunk(c):
                sl = csl(c)
                w_prev = w_state[(c + 1) % 2]
                w_next = w_state[c % 2]
                if c > 0:
                    pkw = pch.tile([128, R], FP, tag="chain")
                    nc.tensor.matmul(pkw[:], lhsT=phikT[:, 0, sl],
                                     rhs=w_prev[:, 0, :], start=True, stop=False)
                    nc.tensor.matmul(pkw[:], lhsT=phikT[:, 1, sl],
                                     rhs=w_prev[:, 1, :], start=False, stop=True)
                    x_t = ps.tile([128, R], FP, tag="xrhs")
                    nc.vector.scalar_tensor_tensor(
                        out=x_t[:], in0=pkw[:], scalar=nbdf[:, c:c + 1],
                        in1=bV[:, c, :], op0=ALU.mult, op1=ALU.add)
                else:
                    x_t = None
                pdm = pch.tile([128, R], FP, tag="chain")
                nc.tensor.matmul(pdm[:], lhsT=JTs[:, c, :],
                                 rhs=(x_t[:] if c > 0 else bV[:, c, :]),
                                 start=True, stop=True)
                dm = ps.tile([128, R], FP, tag="dm")
                nc.vector.tensor_copy(dm[:], pdm[:])
                # retrieved = df^i QW0 + A @ Dm
                po = pch.tile([128, R], FP, tag="chain")
                if c > 0:
                    nc.tensor.matmul(po[:], lhsT=phiqTs[:, 0, sl],
                                     rhs=w_prev[:, 0, :], start=True, stop=False)
                    nc.tensor.matmul(po[:], lhsT=phiqTs[:, 1, sl],
                                     rhs=w_prev[:, 1, :], start=False, stop=False)
                    nc.tensor.matmul(po[:], lhsT=ATs[:, c, :], rhs=dm[:],
                                     start=False, stop=True)
                else:
                    nc.tensor.matmul(po[:], lhsT=ATs[:, c, :], rhs=dm[:],
                                     start=True, stop=True)
                nc.vector.tensor_copy(ys[:, c, :], po[:])
                # W update: W_new^T = dfC * W_prev^T + K'^T @ Dm
                for kt in range(KT):
                    pw = pch.tile([128, R], FP, tag="chain")
                    nc.tensor.matmul(pw[:], lhsT=kps[:, c, kt * 128:(kt + 1) * 128],
                                     rhs=dm[:], start=True, stop=True)
                    if c > 0:
                        nc.vector.scalar_tensor_tensor(
                            out=w_next[:, kt, :], in0=w_prev[:, kt, :],
                            scalar=float(dfC), in1=pw[:],
                            op0=ALU.mult, op1=ALU.add)
                    else:
                        nc.vector.tensor_copy(w_next[:, kt, :], pw[:])
                # final-LN partial stats for this chunk
                nc.vector.reduce_sum(out=s1[:, c:c + 1], in_=ys[:, c, :], axis=AX)
                sq_t = ps.tile([128, R], FP, tag="sq")
                nc.scalar.activation(sq_t[:], ys[:, c, :], AF.Square,
                                     accum_out=s2[:, c:c + 1])

            # ---------------- emission ----------------
            # staggered so PE always has later-stage work queued behind the
            # sequential chain: proj(c+2), prep(c+1), chain(c)
            proj_chunk(0)
            proj_chunk(1)
            prep_chunk(0)
            for c in range(NCH):
                if c + 2 < NCH:
                    proj_chunk(c + 2)
                if c + 1 < NCH:
                    prep_chunk(c + 1)
                chain_chunk(c)

            # ---------------- collective: full-D LN stats ----------------
            cc_in = pd.tile([2, L], FP)
            cc_out = pd.tile([2, L], FP)
            nc.gpsimd.dma_start(out=cc_in[0, :].rearrange("(c p) -> p c", p=128),
                              in_=s1[:, :])
            nc.gpsimd.dma_start(out=cc_in[1, :].rearrange("(c p) -> p c", p=128),
                              in_=s2[:, :])
            nc.gpsimd.collective_compute(
                "AllReduce", ALU.add, replica_groups=REPLICA_GROUPS,
                ins=[cc_in.opt()], outs=[cc_out.opt()])
            s1b = ps2.tile([128, NCH], FP, tag="s1b")
            s2b = ps2.tile([128, NCH], FP, tag="s2b")
            nc.gpsimd.dma_start(out=s1b[:], in_=cc_out[0, :].rearrange(
                "(c p) -> p c", p=128))
            nc.gpsimd.dma_start(out=s2b[:], in_=cc_out[1, :].rearrange(
                "(c p) -> p c", p=128))
            mu = ps2.tile([128, NCH], FP, tag="mu")
            rstd = ps2.tile([128, NCH], FP, tag="rstd")
            var = ps2.tile([128, NCH], FP, tag="var")
            nc.vector.tensor_scalar_mul(mu[:], s1b[:], 1.0 / D)
            nc.vector.tensor_scalar_mul(var[:], s2b[:], 1.0 / D)
            m2 = ps2.tile([128, NCH], FP, tag="m2")
            nc.vector.tensor_mul(m2[:], mu[:], mu[:])
            nc.vector.tensor_sub(var[:], var[:], m2[:])
            nc.scalar.activation(var[:], var[:], AF.Sqrt, bias=eps_t[:])
            nc.vector.reciprocal(rstd[:], var[:])

            # ---------------- final: normalize + partial out proj ----------
            out_ap = out_d[:, :].rearrange("(c p) d -> p c d", p=128)
            for c in range(NCH):
                yn = ps.tile([128, R], FP, tag="yn")
                nc.vector.tensor_scalar(
                    out=yn[:], in0=ys[:, c, :], scalar1=mu[:, c:c + 1],
                    scalar2=rstd[:, c:c + 1], op0=ALU.subtract, op1=ALU.mult)
                nc.vector.tensor_mul(yn[:], yn[:], lngB[:])
                nc.vector.tensor_add(yn[:], yn[:], lnbB[:])
                ptp = ppr.tile([128, 128], FP, tag="prep")
                nc.tensor.transpose(ptp[:], yn[:], ident[:])
                ynT = ps.tile([128, R], FP, tag="ynT")
                nc.vector.tensor_copy(ynT[:], ptp[:])
                pf = pch.tile([128, D], FP, tag="chain")
                nc.tensor.matmul(pf[:], lhsT=ynT[:], rhs=wo[:], start=True,
                                 stop=True)
                ostg = ps.tile([128, D], FP, tag="ostg")
                nc.vector.tensor_copy(ostg[:], pf[:])
                nc.gpsimd.dma_start(out=out_ap[:, c, :], in_=ostg[:])

    nc.compile()
    return nc


def kernel(**inputs):
    x = np.ascontiguousarray(np.asarray(inputs["x"], np.float32))
    Wq = np.asarray(inputs["Wq"], np.float32)
    Wk = np.asarray(inputs["Wk"], np.float32)
    Wv = np.asarray(inputs["Wv"], np.float32)
    beta_w = np.asarray(inputs["beta_w"], np.float32)
    beta_b = np.asarray(inputs["beta_b"], np.float32)
    decay = np.asarray(inputs["decay"], np.float32)
    Wo = np.asarray(inputs["Wo"], np.float32)
    bo = np.asarray(inputs["bo"], np.float32)
    ln_g = np.asarray(inputs["ln_g"], np.float32)
    ln_b = np.asarray(inputs["ln_b"], np.float32)
    lnp_g = np.asarray(inputs["lnp_g"], np.float32)
    lnp_b = np.asarray(inputs["lnp_b"], np.float32)

    df = float(1.0 / (1.0 + np.exp(-float(decay[0]))))
    dfC = df ** C
    consts = _host_consts(df, Wq, Wk, beta_w, lnp_g, lnp_b)
    nc = _build(df, dfC, float(beta_b[0]), consts)

    in_maps = []
    for b in range(B):
        xT = np.ascontiguousarray(x[b].T.reshape(KT, 128, L).transpose(1, 0, 2))
        for h in range(2):
            rs = slice(h * R, (h + 1) * R)
            in_maps.append({
                "xT": xT,
                "wvT": np.ascontiguousarray(
                    Wv[rs, :].T.reshape(KT, 128, R).transpose(1, 0, 2)),
                "woT": np.ascontiguousarray(Wo[:, rs].T),
                "lngb": np.stack([ln_g[rs], ln_b[rs]]).astype(np.float32),
            })

    res = run_bass_kernel_spmd(nc, in_maps, core_ids=list(range(2 * B)),
                               **_RUN_KWARGS)
    globals()["_last_results"] = res
    out = np.zeros((B, L, D), np.float32)
    for b in range(B):
        out[b] = res.results[2 * b]["out_part"] + res.results[2 * b + 1]["out_part"]
        out[b] += bo[None, :]
    return out
